# revision 19
# baseline (speedup 1.0000x reference)
"""Trainium2 Bass kernel for windowed (sparse) attention — v3.

Module: LayerNorm -> overlapping 8x8 spatial windows (stride 6) over a
[2,2,128,128,256] image -> per-window 8-head attention over L=128 tokens
(t*8*8) -> output projection -> overlap-add with count normalization.

Strategy: 882 independent windows sharded over 8 cores (112 each, padded
to 896).  Host does im2win gather + overlap-add scatter; all model
compute runs on device.

Performance history: v1 (DMA-rearrange design) 1817 us -> final 339 us.
Key design points:
  - fp16 DRAM I/O; LN stats via bn_stats/bn_aggr on the DVE
  - 1/sqrt(var+eps) = Exp(-0.5*Ln(var+eps)) on the scalar engine, batched
    over 4-window groups; all ACT functions (Exp/Ln/Copy) are forced into
    the single `natural_log_exp_and_others` table set (see _patched_gat)
    so the scalar engine never swaps activation tables (~2.7us per swap)
  - xn -> xnT via PE transpose (is_transpose matmul), no DMA transposes
  - S matmuls use K=128 with a zero-padded head-major q ("qz" built by a
    single gpsimd masked multiply: stride-0 replicated read x 0/1 mask);
    k chunks feed lhsT straight from the projection layout, and matmul
    columns being independent lets 4 heads share one N=512 instruction
  - softmax denominators as 4 ones-matmuls of N=256 via 3-dim rhs APs
  - normalize via reciprocal_approx_fast + scalar_tensor_tensor (DVE ops
    may read at most ONE PSUM operand; divide is not an ISA op)
  - 14-deep software pipeline: every cross-engine hop (PE->ACT/DVE/Pool
    evacuations, qz build, store) runs >= 1 iteration after its producer
    so in-order engine queues never block mid-iteration; PSUM uses
    exactly 8 banks (tp 1 | qkp 1 | vp 1 | sp 2 | d/op 2 | zp 1)
"""

import functools
import math
from contextlib import ExitStack

import numpy as np

import concourse.bacc as bacc
import concourse.bass as bass
import concourse.mybir as mybir
import concourse.tile as tile
from concourse.bass import AP
from concourse.bass_utils import run_bass_kernel_spmd

# Force every ACT function this kernel uses (Exp, Ln, Copy) into the single
# combined table set `natural_log_exp_and_others` so the scalar engine never
# swaps activation tables (each swap costs ~2.7us).  Indices into
# act_info.json must be preserved, so we strip these functions from every
# other set rather than reordering.
_ORIG_GAT = bacc.get_activation_tables

def _patched_gat(arch):
    tabs = _ORIG_GAT(arch)
    strip = {mybir.ActivationFunctionType.from_pwp(n)
             for n in ("exp", "ln", "copy", "identity")}
    return {n: (fns if n == "natural_log_exp_and_others" else fns - strip)
            for n, fns in tabs.items()}

bacc.get_activation_tables = _patched_gat

# Problem constants (hardcoded per contract - kernel.py is self-contained).
B, T, H, W, C = 2, 2, 128, 128, 256
MID, HEADS = 256, 8
HD = MID // HEADS          # 32
PATCH, STEP = 8, 6         # window size / stride
NHW = 21                   # windows per axis: starts 0,6,...,120
NWIN = NHW * NHW * B       # 882 flat windows (n outer, b inner)
L = T * PATCH * PATCH      # 128 tokens per window
NCORES = 8
NW = 112                   # windows per core after padding to 896
EPS = 1e-6
F32, F16 = mybir.dt.float32, mybir.dt.float16
AF = mybir.ActivationFunctionType
ALU = mybir.AluOpType


def _build_program(nw: int):
    nc = bacc.Bacc(
        "TRN2",
        target_bir_lowering=False,
        debug=False,
        enable_asserts=False,
        num_devices=NCORES,
    )
    xw = nc.dram_tensor("xw", [nw * 128, 256], F16, kind="ExternalInput").ap()
    wq = nc.dram_tensor("wq", [256, 256], F16, kind="ExternalInput").ap()
    wk = nc.dram_tensor("wk", [256, 256], F16, kind="ExternalInput").ap()
    wv = nc.dram_tensor("wv", [256, 256], F16, kind="ExternalInput").ap()
    wo = nc.dram_tensor("wo", [256, 256], F16, kind="ExternalInput").ap()
    ones1 = nc.dram_tensor("ones1", [128, 32], F16, kind="ExternalInput").ap()
    ident = nc.dram_tensor("ident", [128, 128], F16, kind="ExternalInput").ap()
    maskq = nc.dram_tensor("maskq", [128, 1024], F16, kind="ExternalInput").ap()
    zt = nc.dram_tensor("zt", [nw * 256, 128], F16, kind="ExternalOutput").ap()

    inv_sqrt_hd = 1.0 / math.sqrt(HD)
    GROUPS = nw // 4
    assert nw % 4 == 0

    with tile.TileContext(nc) as tc, ExitStack() as ctx:
        pw = ctx.enter_context(tc.tile_pool(name="wts", bufs=1))
        wq_s = [pw.tile([128, 256], F16, tag=f"wq{i}", name=f"wq{i}") for i in range(2)]
        wk_s = [pw.tile([128, 256], F16, tag=f"wk{i}", name=f"wk{i}") for i in range(2)]
        wv_s = [pw.tile([128, 256], F16, tag=f"wv{i}", name=f"wv{i}") for i in range(2)]
        wo_s = [pw.tile([128, 256], F16, tag=f"wo{i}", name=f"wo{i}") for i in range(2)]
        for i in range(2):
            nc.sync.dma_start(wq_s[i][:], wq[i * 128:(i + 1) * 128, :])
            nc.sync.dma_start(wk_s[i][:], wk[i * 128:(i + 1) * 128, :])
            nc.sync.dma_start(wv_s[i][:], wv[i * 128:(i + 1) * 128, :])
            nc.sync.dma_start(wo_s[i][:], wo[i * 128:(i + 1) * 128, :])
        ones_s = pw.tile([128, 32], F16, tag="ones1", name="ones1")
        nc.sync.dma_start(ones_s[:], ones1)
        id_s = pw.tile([128, 128], F16, tag="ident", name="ident")
        nc.sync.dma_start(id_s[:], ident)
        mq_s = pw.tile([128, 1024], F16, tag="maskq", name="maskq")
        nc.sync.dma_start(mq_s[:], maskq)
        eps_s = pw.tile([128, 1], F32, tag="eps", name="eps")
        nc.vector.memset(eps_s[:], EPS)

        # SBUF pools
        pxt = ctx.enter_context(tc.tile_pool(name="pxt", bufs=4))
        pxn = ctx.enter_context(tc.tile_pool(name="pxn", bufs=3))
        pxnt = ctx.enter_context(tc.tile_pool(name="pxnt", bufs=3))
        pqks = ctx.enter_context(tc.tile_pool(name="pqks", bufs=4))
        pqz = ctx.enter_context(tc.tile_pool(name="pqz", bufs=4))
        pvs = ctx.enter_context(tc.tile_pool(name="pvs", bufs=6))
        pes = ctx.enter_context(tc.tile_pool(name="pes", bufs=4))
        pos = ctx.enter_context(tc.tile_pool(name="pos", bufs=3))
        pzs = ctx.enter_context(tc.tile_pool(name="pzs", bufs=3))
        pst = ctx.enter_context(tc.tile_pool(name="pst", bufs=3))
        pch = ctx.enter_context(tc.tile_pool(name="pch", bufs=2))
        # PSUM pools: 1 + 1 + 1 + 2 + 3 = 8 banks
        ptp = ctx.enter_context(tc.tile_pool(name="ptp", bufs=1, space="PSUM"))
        pqk = ctx.enter_context(tc.tile_pool(name="pqk", bufs=1, space="PSUM"))
        pv = ctx.enter_context(tc.tile_pool(name="pv", bufs=1, space="PSUM"))
        psp = ctx.enter_context(tc.tile_pool(name="psp", bufs=2, space="PSUM"))
        pd = ctx.enter_context(tc.tile_pool(name="pd", bufs=2, space="PSUM"))
        pz = ctx.enter_context(tc.tile_pool(name="pz", bufs=1, space="PSUM"))

        # Pipeline state
        xt_g = {}
        bag_g, rs4_g = {}, {}
        xn_t, xnt_t, qks_t, qz_t, vs_t, es_t, os_t = {}, {}, {}, {}, {}, {}, {}
        dp_t, zp_t, spa_t, spb_t = {}, {}, {}, {}

        def load_group(g):
            t = pxt.tile([128, 1024], F16, tag="xt", name="xt")
            src = AP(xw.tensor, g * 4 * 128 * 256,
                     [[256, 128], [128 * 256, 4], [1, 256]])
            nc.sync.dma_start(t[:], src)
            xt_g[g] = t
            xt_g.pop(g - 3, None)

        load_group(0)

        for i in range(nw + 16):
            if (i + 2) % 4 == 0:
                g = (i + 2) // 4
                if g < GROUPS:
                    load_group(g)

            # ---- s0 (w=i): LN stats into group tile ----
            w = i
            if w < nw:
                g = w // 4
                if w % 4 == 0:
                    bag_g[g] = pst.tile([128, 8], F32, tag="bagg", name="bagg")
                xt = xt_g[g]
                xs = xt[:, (w % 4) * 256:(w % 4 + 1) * 256]
                bst = pst.tile([128, 6], F32, tag="bst", name="bst")
                nc.vector.bn_stats(bst[:], xs)
                nc.vector.bn_aggr(bag_g[g][:, 2 * (w % 4):2 * (w % 4) + 2], bst[:])

            # ---- group rsqrt: rs4 = Exp(-0.5*Ln(var4+eps)), 4 windows/op ----
            if i % 4 == 0 and i // 4 - 1 >= 0 and i // 4 - 1 < GROUPS:
                g = i // 4 - 1
                bag = bag_g[g]
                var4 = AP(bag[:].tensor, bag[:].offset + 1, [[8, 128], [2, 4]])
                lnv = pch.tile([128, 4], F32, tag="lnv", name="lnv")
                nc.scalar.activation(lnv[:], var4, AF.Ln, bias=eps_s[:])
                rs4 = pch.tile([128, 4], F32, tag="rs4", name="rs4")
                nc.scalar.activation(rs4[:], lnv[:], AF.Exp, scale=-0.5)
                rs4_g[g] = rs4

            # ---- s_apply (w=i-5): LN apply -> xn f16 ----
            w = i - 5
            if 0 <= w < nw:
                g = w // 4
                xt = xt_g[g]
                xs = xt[:, (w % 4) * 256:(w % 4 + 1) * 256]
                mu_w = pst.tile([128, 1], F32, tag="muw", name="muw")
                nc.vector.tensor_copy(mu_w[:],
                                      bag_g[g][:, 2 * (w % 4):2 * (w % 4) + 1])
                rs_w = pst.tile([128, 1], F32, tag="rsw", name="rsw")
                nc.vector.tensor_copy(rs_w[:], rs4_g[g][:, w % 4:w % 4 + 1])
                xn = pxn.tile([128, 256], F16, tag="xn", name="xn")
                nc.vector.tensor_scalar(
                    out=xn[:], in0=xs, scalar1=mu_w[:], scalar2=rs_w[:],
                    op0=ALU.subtract, op1=ALU.mult,
                )
                xn_t[w] = xn
                if w % 4 == 3:
                    bag_g.pop(g, None)
                    rs4_g.pop(g, None)

            # ---- s_T (w=i-6): PE transpose + evac ----
            w = i - 6
            if 0 <= w < nw:
                tp = ptp.tile([128, 1024], F16, tag="tp", name="tp")
                for kc in range(2):
                    nc.tensor.transpose(
                        tp[:, kc * 128:(kc + 1) * 128],
                        xn_t[w][:, kc * 128:(kc + 1) * 128], id_s[:])
                xnt = pxnt.tile([128, 256], F16, tag="xnt", name="xnt")
                nc.vector.tensor_copy(xnt[:], tp[:, 0:256])
                xnt_t[w] = xnt
                del xn_t[w]

            # ---- s_expa (w=i-10): exp of heads 0-3 (sp_a from last iter) --
            w = i - 10
            if 0 <= w < nw:
                es = pes.tile([128, 1024], F16, tag="es", name="es")
                nc.scalar.activation(es[:, 0:512], spa_t[w][:], AF.Exp,
                                     scale=inv_sqrt_hd)
                es_t[w] = es

            # ---- s_expb (w=i-11): exp of heads 4-7 ----
            w = i - 11
            if 0 <= w < nw:
                nc.scalar.activation(es_t[w][:, 512:1024], spb_t[w][:], AF.Exp,
                                     scale=inv_sqrt_hd)
                del spa_t[w], spb_t[w]

            # ---- s_qkv (w=i-7): projections + evacs + qz ----
            w = i - 7
            if 0 <= w < nw:
                xnt = xnt_t[w]
                qkp = pqk.tile([128, 512], F32, tag="qk", name="qk")
                first = True
                for ws, base in ((wq_s, 0), (wk_s, 256)):
                    for mh in range(2):
                        for kc in range(2):
                            nc.tensor.matmul(
                                qkp[:, base + mh * 128:base + (mh + 1) * 128],
                                lhsT=ws[kc][:, mh * 128:(mh + 1) * 128],
                                rhs=xnt[:, kc * 128:(kc + 1) * 128],
                                start=first,
                                stop=(base == 256 and mh == 1 and kc == 1),
                            )
                            first = False
                vp = pv.tile([128, 512], F32, tag="v", name="v")
                for kc in range(2):
                    nc.tensor.matmul(
                        vp[:, 0:256], lhsT=xnt[:, kc * 128:(kc + 1) * 128],
                        rhs=wv_s[kc][:], start=(kc == 0), stop=(kc == 1),
                    )
                qks = pqks.tile([128, 512], F16, tag="qks", name="qks")
                nc.scalar.copy(qks[:], qkp[:])
                vs = pvs.tile([128, 256], F16, tag="vs", name="vs")
                nc.vector.tensor_copy(vs[:], vp[:, 0:256])
                qks_t[w], vs_t[w] = qks, vs
                del xnt_t[w]

            # ---- s_qz (w=i-8): build zero-padded head-major q on gpsimd ----
            w = i - 8
            if 0 <= w < nw:
                qks = qks_t[w]
                qz = pqz.tile([128, 1024], F16, tag="qz", name="qz")
                qrep = AP(qks[:].tensor, qks[:].offset,
                          [[512, 128], [128, 2], [0, 4], [1, 128]])
                nc.gpsimd.tensor_tensor(out=qz[:], in0=qrep, in1=mq_s[:],
                                        op=ALU.mult)
                qz_t[w] = qz

            # ---- s_Sa (w=i-9): S matmul, heads 0-3 ----
            w = i - 9
            if 0 <= w < nw:
                spa = psp.tile([128, 512], F32, tag="sp", name="sp")
                nc.tensor.matmul(
                    spa[:], lhsT=qks_t[w][:, 256:384], rhs=qz_t[w][:, 0:512],
                    start=True, stop=True,
                )
                spa_t[w] = spa

            # ---- s_Sb (w=i-10): S matmul, heads 4-7 ----
            w = i - 10
            if 0 <= w < nw:
                spb = psp.tile([128, 512], F32, tag="sp", name="sp")
                nc.tensor.matmul(
                    spb[:], lhsT=qks_t[w][:, 384:512], rhs=qz_t[w][:, 512:1024],
                    start=True, stop=True,
                )
                spb_t[w] = spb
                del qks_t[w], qz_t[w]

            # ---- s_DOT (w=i-12): D (4x N=256) + OT (8x) matmuls ----
            w = i - 12
            if 0 <= w < nw:
                es, vs = es_t[w], vs_t[w]
                dp = pd.tile([128, 512], F32, tag="dp", name="dp")
                for j in range(4):
                    rhs = AP(es[:].tensor, es[:].offset + j * 128,
                             [[1024, 128], [512, 2], [1, 128]])
                    nc.tensor.matmul(
                        dp[32 * j:32 * j + 32, 0:256],
                        lhsT=ones_s[:], rhs=rhs,
                        start=True, stop=True, tile_position=(0, 32 * j),
                        skip_group_check=True,
                    )
                op_ = pd.tile([128, 512], F32, tag="dp", name="dp")
                for h in range(HEADS):
                    r, j = h // 4, h % 4
                    nc.tensor.matmul(
                        op_[32 * j:32 * j + 32, r * 128:(r + 1) * 128],
                        lhsT=vs[:, 32 * h:32 * h + 32],
                        rhs=es[:, h * 128:(h + 1) * 128],
                        start=(r == 0), stop=(r == 1), tile_position=(0, 32 * j),
                        skip_group_check=True,
                    )
                dp_t[w] = (dp, op_)
                del es_t[w], vs_t[w]

            # ---- s_norm (w=i-13): softmax normalize on DVE ----
            w = i - 13
            if 0 <= w < nw:
                dp, op_ = dp_t[w]
                dbs = pst.tile([128, 256], F32, tag="dbs", name="dbs")
                nc.vector.reciprocal_approx_fast(out=dbs[:], in_=dp[:, 0:256])
                os_ = pos.tile([128, 256], F16, tag="os", name="os")
                nc.vector.scalar_tensor_tensor(
                    out=os_[:], in0=op_[:, 0:256], scalar=1.0, in1=dbs[:],
                    op0=ALU.mult, op1=ALU.mult,
                )
                os_t[w] = os_
                del dp_t[w]

            # ---- s_Z (w=i-14): out projection ----
            w = i - 14
            if 0 <= w < nw:
                os_ = os_t[w]
                zp = pz.tile([128, 512], F32, tag="zp", name="zp")
                for coh in range(2):
                    for kc in range(2):
                        nc.tensor.matmul(
                            zp[:, coh * 128:(coh + 1) * 128],
                            lhsT=wo_s[kc][:, coh * 128:(coh + 1) * 128],
                            rhs=os_[:, kc * 128:(kc + 1) * 128],
                            start=(coh == 0 and kc == 0),
                            stop=(coh == 1 and kc == 1),
                        )
                zp_t[w] = zp
                del os_t[w]

            # ---- s_zs (w=i-15): evac + store ----
            w = i - 15
            if 0 <= w < nw:
                zp = zp_t[w]
                zs = pzs.tile([128, 256], F16, tag="zs", name="zs")
                nc.scalar.copy(zs[:], zp[:, 0:256])
                dst = AP(zt.tensor, w * 256 * 128,
                         [[128, 128], [128 * 128, 2], [1, 128]])
                nc.sync.dma_start(dst, zs[:])
                del zp_t[w]
    nc.compile()
    return nc


@functools.lru_cache(maxsize=2)
def _get_program(nw: int):
    return _build_program(nw)


def _im2win(x: np.ndarray) -> np.ndarray:
    """[B,T,H,W,C] -> [882,128,256] windows, flat order f = i_n*B + i_b."""
    s = x.strides
    xs = np.lib.stride_tricks.as_strided(
        x,
        shape=(B, T, NHW, PATCH, NHW, PATCH, C),
        strides=(s[0], s[1], STEP * s[2], s[2], STEP * s[3], s[3], s[4]),
    )
    w = xs.transpose(2, 4, 0, 1, 3, 5, 6)  # [iH,iW,b,t,p,q,c]
    return np.ascontiguousarray(w.reshape(NHW * NHW * B, L, C))


def _overlap_add(zwin: np.ndarray, bo: np.ndarray) -> np.ndarray:
    """[882,128,256] window outputs -> [B,T,H,W,C] with count-normalize + bo."""
    th = np.arange(NHW) * STEP
    z = zwin.reshape(B, NHW, NHW, T, PATCH, PATCH, MID)  # [b,iH,iW,t,p,q,c]
    acc = np.zeros((B, T, H, W, MID), np.float32)
    count = np.zeros((H, W), np.float32)
    for p in range(PATCH):
        rid = (th + p)[:, None]
        for q in range(PATCH):
            cid = (th + q)[None, :]
            acc[:, :, rid, cid, :] += z[:, :, :, :, p, q, :].transpose(0, 3, 1, 2, 4)
            count[rid, cid] += 1.0
    out = acc / count[None, None, :, :, None] + bo[None, None, None, None, :]
    return out


LAST_RESULT = None


def kernel(x, ln_g, ln_b, Wq, Wk, Wv, Wo, bo):
    x = np.asarray(x, np.float32)
    ln_g = np.asarray(ln_g, np.float32)
    ln_b = np.asarray(ln_b, np.float32)
    assert np.allclose(ln_b, 0.0), "kernel folds ln_g into weights; ln_b must be 0"
    # Fold LN gamma into the input side of Wq/Wk/Wv.
    wq_t = np.ascontiguousarray((np.asarray(Wq, np.float32) * ln_g).T.astype(np.float16))
    wk_t = np.ascontiguousarray((np.asarray(Wk, np.float32) * ln_g).T.astype(np.float16))
    wv_t = np.ascontiguousarray((np.asarray(Wv, np.float32) * ln_g).T.astype(np.float16))
    wo_t = np.ascontiguousarray(np.asarray(Wo, np.float32).T.astype(np.float16))
    ones1 = np.ones((128, 32), np.float16)
    ident = np.eye(128, dtype=np.float16)
    maskq = np.zeros((128, 1024), np.float16)
    for h in range(HEADS):
        j = h % 4
        maskq[32 * j:32 * j + 32, h * 128:(h + 1) * 128] = 1.0

    win = _im2win(x)                              # [882, 128, 256]
    pad = NCORES * NW - NWIN                      # 14
    winp = np.concatenate([win, np.zeros((pad, L, C), np.float32)], 0)
    shards = winp.reshape(NCORES, NW * L, C)

    nc = _get_program(NW)
    trace = bool(int(__import__("os").environ.get("KERNEL_TRACE", "0")))
    in_maps = []
    for i in range(NCORES):
        in_maps.append({
            "xw": np.ascontiguousarray(shards[i]).astype(np.float16),
            "wq": wq_t, "wk": wk_t, "wv": wv_t, "wo": wo_t,
            "ones1": ones1, "ident": ident, "maskq": maskq,
        })
    res = run_bass_kernel_spmd(nc, in_maps, core_ids=list(range(NCORES)),
                               trace=trace)
    global LAST_RESULT
    LAST_RESULT = res
    zts = [np.asarray(res.results[i]["zt"], np.float32).reshape(NW, 2, 128, 128)
           for i in range(NCORES)]
    # zt rows: w*256 + c_out, cols l  ->  Z_w[l, c] = zt[w, :, :, l]
    zall = np.concatenate(zts, 0)                 # [896, 2, 128, 128]
    zwin = zall.reshape(NCORES * NW, MID, L).transpose(0, 2, 1)[:NWIN]
    return _overlap_add(np.ascontiguousarray(zwin), np.asarray(bo, np.float32))


# revision 23
# speedup vs baseline: 1.3660x; 1.3660x over previous
"""Trainium2 Bass kernel for windowed (sparse) attention — v3.

Module: LayerNorm -> overlapping 8x8 spatial windows (stride 6) over a
[2,2,128,128,256] image -> per-window 8-head attention over L=128 tokens
(t*8*8) -> output projection -> overlap-add with count normalization.

Strategy: 882 independent windows sharded over 8 cores (112 each, padded
to 896).  Host does im2win gather + overlap-add scatter; all model
compute runs on device.

Performance history: v1 (DMA-rearrange design) 1817 us -> final 339 us.
Key design points:
  - fp16 DRAM I/O; LN stats via bn_stats/bn_aggr on the DVE
  - 1/sqrt(var+eps) = Exp(-0.5*Ln(var+eps)) on the scalar engine, batched
    over 4-window groups; all ACT functions (Exp/Ln/Copy) are forced into
    the single `natural_log_exp_and_others` table set (see _patched_gat)
    so the scalar engine never swaps activation tables (~2.7us per swap)
  - xn -> xnT via PE transpose (is_transpose matmul), no DMA transposes
  - S matmuls use K=128 with a zero-padded head-major q ("qz" built by a
    single gpsimd masked multiply: stride-0 replicated read x 0/1 mask);
    k chunks feed lhsT straight from the projection layout, and matmul
    columns being independent lets 4 heads share one N=512 instruction
  - softmax denominators as 4 ones-matmuls of N=256 via 3-dim rhs APs
  - normalize via reciprocal_approx_fast + scalar_tensor_tensor (DVE ops
    may read at most ONE PSUM operand; divide is not an ISA op)
  - 14-deep software pipeline: every cross-engine hop (PE->ACT/DVE/Pool
    evacuations, qz build, store) runs >= 1 iteration after its producer
    so in-order engine queues never block mid-iteration; PSUM uses
    exactly 8 banks (tp 1 | qkp 1 | vp 1 | sp 2 | d/op 2 | zp 1)
"""

import functools
import math
from contextlib import ExitStack

import numpy as np

import concourse.bacc as bacc
import concourse.bass as bass
import concourse.mybir as mybir
import concourse.tile as tile
from concourse.bass import AP
from concourse.bass_utils import run_bass_kernel_spmd

# Force every ACT function this kernel uses (Exp, Ln, Copy) into the single
# combined table set `natural_log_exp_and_others` so the scalar engine never
# swaps activation tables (each swap costs ~2.7us).  Indices into
# act_info.json must be preserved, so we strip these functions from every
# other set rather than reordering.
_ORIG_GAT = bacc.get_activation_tables

def _patched_gat(arch):
    tabs = _ORIG_GAT(arch)
    strip = {mybir.ActivationFunctionType.from_pwp(n)
             for n in ("exp", "ln", "copy", "identity")}
    return {n: (fns if n == "natural_log_exp_and_others" else fns - strip)
            for n, fns in tabs.items()}

bacc.get_activation_tables = _patched_gat

# Problem constants (hardcoded per contract - kernel.py is self-contained).
B, T, H, W, C = 2, 2, 128, 128, 256
MID, HEADS = 256, 8
HD = MID // HEADS          # 32
PATCH, STEP = 8, 6         # window size / stride
NHW = 21                   # windows per axis: starts 0,6,...,120
NWIN = NHW * NHW * B       # 882 flat windows (n outer, b inner)
L = T * PATCH * PATCH      # 128 tokens per window
NCORES = 8
NW = 112                   # windows per core after padding to 896
EPS = 1e-6
F32, F16 = mybir.dt.float32, mybir.dt.float16
AF = mybir.ActivationFunctionType
ALU = mybir.AluOpType


def _build_program(nw: int):
    nc = bacc.Bacc(
        "TRN2",
        target_bir_lowering=False,
        debug=False,
        enable_asserts=False,
        num_devices=NCORES,
    )
    xw = nc.dram_tensor("xw", [nw * 128, 256], F16, kind="ExternalInput").ap()
    wq = nc.dram_tensor("wq", [256, 256], F16, kind="ExternalInput").ap()
    wk = nc.dram_tensor("wk", [256, 256], F16, kind="ExternalInput").ap()
    wv = nc.dram_tensor("wv", [256, 256], F16, kind="ExternalInput").ap()
    wo = nc.dram_tensor("wo", [256, 256], F16, kind="ExternalInput").ap()
    ones1 = nc.dram_tensor("ones1", [128, 32], F16, kind="ExternalInput").ap()
    ident = nc.dram_tensor("ident", [128, 128], F16, kind="ExternalInput").ap()
    maskq = nc.dram_tensor("maskq", [128, 1024], F16, kind="ExternalInput").ap()
    zt = nc.dram_tensor("zt", [nw * 256, 128], F16, kind="ExternalOutput").ap()

    inv_sqrt_hd = 1.0 / math.sqrt(HD)
    GROUPS = nw // 4
    assert nw % 4 == 0

    with tile.TileContext(nc) as tc, ExitStack() as ctx:
        pw = ctx.enter_context(tc.tile_pool(name="wts", bufs=1))
        wq_s = [pw.tile([128, 256], F16, tag=f"wq{i}", name=f"wq{i}") for i in range(2)]
        wk_s = [pw.tile([128, 256], F16, tag=f"wk{i}", name=f"wk{i}") for i in range(2)]
        wv_s = [pw.tile([128, 256], F16, tag=f"wv{i}", name=f"wv{i}") for i in range(2)]
        wo_s = [pw.tile([128, 256], F16, tag=f"wo{i}", name=f"wo{i}") for i in range(2)]
        for i in range(2):
            nc.sync.dma_start(wq_s[i][:], wq[i * 128:(i + 1) * 128, :])
            nc.sync.dma_start(wk_s[i][:], wk[i * 128:(i + 1) * 128, :])
            nc.sync.dma_start(wv_s[i][:], wv[i * 128:(i + 1) * 128, :])
            nc.sync.dma_start(wo_s[i][:], wo[i * 128:(i + 1) * 128, :])
        ones_s = pw.tile([128, 32], F16, tag="ones1", name="ones1")
        nc.sync.dma_start(ones_s[:], ones1)
        id_s = pw.tile([128, 128], F16, tag="ident", name="ident")
        nc.sync.dma_start(id_s[:], ident)
        mq_s = pw.tile([128, 1024], F16, tag="maskq", name="maskq")
        nc.sync.dma_start(mq_s[:], maskq)
        eps_s = pw.tile([128, 1], F32, tag="eps", name="eps")
        nc.vector.memset(eps_s[:], EPS)

        # SBUF pools
        pxt = ctx.enter_context(tc.tile_pool(name="pxt", bufs=4))
        pxn = ctx.enter_context(tc.tile_pool(name="pxn", bufs=3))
        pxnt = ctx.enter_context(tc.tile_pool(name="pxnt", bufs=3))
        pqks = ctx.enter_context(tc.tile_pool(name="pqks", bufs=3))
        pqz = ctx.enter_context(tc.tile_pool(name="pqz", bufs=3))
        pvs = ctx.enter_context(tc.tile_pool(name="pvs", bufs=4))
        pes = ctx.enter_context(tc.tile_pool(name="pes", bufs=3))
        pos = ctx.enter_context(tc.tile_pool(name="pos", bufs=3))
        pzs = ctx.enter_context(tc.tile_pool(name="pzs", bufs=3))
        pst = ctx.enter_context(tc.tile_pool(name="pst", bufs=3))
        pch = ctx.enter_context(tc.tile_pool(name="pch", bufs=2))
        # PSUM pools: 1 + 1 + 1 + 2 + 3 = 8 banks
        ptp = ctx.enter_context(tc.tile_pool(name="ptp", bufs=1, space="PSUM"))
        pqk = ctx.enter_context(tc.tile_pool(name="pqk", bufs=1, space="PSUM"))
        pv = ctx.enter_context(tc.tile_pool(name="pv", bufs=1, space="PSUM"))
        psp = ctx.enter_context(tc.tile_pool(name="psp", bufs=1, space="PSUM"))
        pd = ctx.enter_context(tc.tile_pool(name="pd", bufs=2, space="PSUM"))
        pz = ctx.enter_context(tc.tile_pool(name="pz", bufs=1, space="PSUM"))

        # Pipeline state
        xt_g = {}
        bag_g, rs4_g = {}, {}
        xn_t, xnt_t, qks_t, qz_t, vs_t, es_t, os_t = {}, {}, {}, {}, {}, {}, {}
        dp_t, zp_t = {}, {}

        def load_group(g):
            t = pxt.tile([128, 1024], F16, tag="xt", name="xt")
            src = AP(xw.tensor, g * 4 * 128 * 256,
                     [[256, 128], [128 * 256, 4], [1, 256]])
            nc.sync.dma_start(t[:], src)
            xt_g[g] = t
            xt_g.pop(g - 3, None)

        load_group(0)

        for i in range(nw + 14):
            if (i + 2) % 4 == 0:
                g = (i + 2) // 4
                if g < GROUPS:
                    load_group(g)

            # ---- s0 (w=i): LN stats into group tile ----
            w = i
            if w < nw:
                g = w // 4
                if w % 4 == 0:
                    bag_g[g] = pst.tile([128, 8], F32, tag="bagg", name="bagg")
                xt = xt_g[g]
                xs = xt[:, (w % 4) * 256:(w % 4 + 1) * 256]
                bst = pst.tile([128, 6], F32, tag="bst", name="bst")
                nc.vector.bn_stats(bst[:], xs)
                nc.vector.bn_aggr(bag_g[g][:, 2 * (w % 4):2 * (w % 4) + 2], bst[:])

            # ---- group rsqrt: rs4 = Exp(-0.5*Ln(var4+eps)), 4 windows/op ----
            if i % 4 == 0 and i // 4 - 1 >= 0 and i // 4 - 1 < GROUPS:
                g = i // 4 - 1
                bag = bag_g[g]
                var4 = AP(bag[:].tensor, bag[:].offset + 1, [[8, 128], [2, 4]])
                lnv = pch.tile([128, 4], F32, tag="lnv", name="lnv")
                nc.scalar.activation(lnv[:], var4, AF.Ln, bias=eps_s[:])
                rs4 = pch.tile([128, 4], F32, tag="rs4", name="rs4")
                nc.scalar.activation(rs4[:], lnv[:], AF.Exp, scale=-0.5)
                rs4_g[g] = rs4

            # ---- s_apply (w=i-5): LN apply -> xn f16 ----
            w = i - 5
            if 0 <= w < nw:
                g = w // 4
                xt = xt_g[g]
                xs = xt[:, (w % 4) * 256:(w % 4 + 1) * 256]
                mu_w = pst.tile([128, 1], F32, tag="muw", name="muw")
                nc.vector.tensor_copy(mu_w[:],
                                      bag_g[g][:, 2 * (w % 4):2 * (w % 4) + 1])
                rs_w = pst.tile([128, 1], F32, tag="rsw", name="rsw")
                nc.vector.tensor_copy(rs_w[:], rs4_g[g][:, w % 4:w % 4 + 1])
                xn = pxn.tile([128, 256], F16, tag="xn", name="xn")
                nc.vector.tensor_scalar(
                    out=xn[:], in0=xs, scalar1=mu_w[:], scalar2=rs_w[:],
                    op0=ALU.subtract, op1=ALU.mult,
                )
                xn_t[w] = xn
                if w % 4 == 3:
                    bag_g.pop(g, None)
                    rs4_g.pop(g, None)

            # ---- s_T (w=i-6): PE transpose + evac ----
            w = i - 6
            if 0 <= w < nw:
                tp = ptp.tile([128, 1024], F16, tag="tp", name="tp")
                for kc in range(2):
                    nc.tensor.transpose(
                        tp[:, kc * 128:(kc + 1) * 128],
                        xn_t[w][:, kc * 128:(kc + 1) * 128], id_s[:])
                xnt = pxnt.tile([128, 256], F16, tag="xnt", name="xnt")
                nc.vector.tensor_copy(xnt[:], tp[:, 0:256])
                xnt_t[w] = xnt
                del xn_t[w]

            # ---- s_qkv (w=i-7): projections + evacs + qz ----
            w = i - 7
            if 0 <= w < nw:
                xnt = xnt_t[w]
                qkp = pqk.tile([128, 512], F32, tag="qk", name="qk")
                first = True
                for ws, base in ((wq_s, 0), (wk_s, 256)):
                    for mh in range(2):
                        for kc in range(2):
                            nc.tensor.matmul(
                                qkp[:, base + mh * 128:base + (mh + 1) * 128],
                                lhsT=ws[kc][:, mh * 128:(mh + 1) * 128],
                                rhs=xnt[:, kc * 128:(kc + 1) * 128],
                                start=first,
                                stop=(base == 256 and mh == 1 and kc == 1),
                            )
                            first = False
                vp = pv.tile([128, 512], F32, tag="v", name="v")
                for kc in range(2):
                    nc.tensor.matmul(
                        vp[:, 0:256], lhsT=xnt[:, kc * 128:(kc + 1) * 128],
                        rhs=wv_s[kc][:], start=(kc == 0), stop=(kc == 1),
                    )
                qks = pqks.tile([128, 512], F16, tag="qks", name="qks")
                nc.scalar.copy(qks[:], qkp[:])
                vs = pvs.tile([128, 256], F16, tag="vs", name="vs")
                nc.vector.tensor_copy(vs[:], vp[:, 0:256])
                qks_t[w], vs_t[w] = qks, vs
                del xnt_t[w]

            # ---- s_qz (w=i-8): build zero-padded head-major q on gpsimd ----
            w = i - 8
            if 0 <= w < nw:
                qks = qks_t[w]
                qz = pqz.tile([128, 1024], F16, tag="qz", name="qz")
                qrep = AP(qks[:].tensor, qks[:].offset,
                          [[512, 128], [128, 2], [0, 4], [1, 128]])
                nc.gpsimd.tensor_tensor(out=qz[:], in0=qrep, in1=mq_s[:],
                                        op=ALU.mult)
                qz_t[w] = qz

            # ---- s_S (w=i-9): S matmuls (4 heads per instr) + exp ----
            w = i - 9
            if 0 <= w < nw:
                qks, qz = qks_t[w], qz_t[w]
                sp = psp.tile([128, 1024], F32, tag="sp", name="sp")
                for mh in range(2):
                    nc.tensor.matmul(
                        sp[:, mh * 512:(mh + 1) * 512],
                        lhsT=qks[:, 256 + mh * 128:256 + (mh + 1) * 128],
                        rhs=qz[:, mh * 512:(mh + 1) * 512],
                        start=True, stop=True,
                    )
                es = pes.tile([128, 1024], F16, tag="es", name="es")
                nc.scalar.activation(es[:], sp[:], AF.Exp, scale=inv_sqrt_hd)
                es_t[w] = es
                del qks_t[w], qz_t[w]

            # ---- s_DOT (w=i-10): D (4x N=256) + OT (8x) matmuls ----
            w = i - 10
            if 0 <= w < nw:
                es, vs = es_t[w], vs_t[w]
                dp = pd.tile([128, 512], F32, tag="dp", name="dp")
                for j in range(4):
                    rhs = AP(es[:].tensor, es[:].offset + j * 128,
                             [[1024, 128], [512, 2], [1, 128]])
                    nc.tensor.matmul(
                        dp[32 * j:32 * j + 32, 0:256],
                        lhsT=ones_s[:], rhs=rhs,
                        start=True, stop=True, tile_position=(0, 32 * j),
                        skip_group_check=True,
                    )
                op_ = pd.tile([128, 512], F32, tag="dp", name="dp")
                for h in range(HEADS):
                    r, j = h // 4, h % 4
                    nc.tensor.matmul(
                        op_[32 * j:32 * j + 32, r * 128:(r + 1) * 128],
                        lhsT=vs[:, 32 * h:32 * h + 32],
                        rhs=es[:, h * 128:(h + 1) * 128],
                        start=(r == 0), stop=(r == 1), tile_position=(0, 32 * j),
                        skip_group_check=True,
                    )
                dp_t[w] = (dp, op_)
                del es_t[w], vs_t[w]

            # ---- s_norm (w=i-11): softmax normalize on DVE ----
            w = i - 11
            if 0 <= w < nw:
                dp, op_ = dp_t[w]
                dbs = pst.tile([128, 256], F32, tag="dbs", name="dbs")
                nc.vector.reciprocal_approx_fast(out=dbs[:], in_=dp[:, 0:256])
                os_ = pos.tile([128, 256], F16, tag="os", name="os")
                nc.vector.scalar_tensor_tensor(
                    out=os_[:], in0=op_[:, 0:256], scalar=1.0, in1=dbs[:],
                    op0=ALU.mult, op1=ALU.mult,
                )
                os_t[w] = os_
                del dp_t[w]

            # ---- s_Z (w=i-12): out projection ----
            w = i - 12
            if 0 <= w < nw:
                os_ = os_t[w]
                zp = pz.tile([128, 512], F32, tag="zp", name="zp")
                for coh in range(2):
                    for kc in range(2):
                        nc.tensor.matmul(
                            zp[:, coh * 128:(coh + 1) * 128],
                            lhsT=wo_s[kc][:, coh * 128:(coh + 1) * 128],
                            rhs=os_[:, kc * 128:(kc + 1) * 128],
                            start=(coh == 0 and kc == 0),
                            stop=(coh == 1 and kc == 1),
                        )
                zp_t[w] = zp
                del os_t[w]

            # ---- s_zs (w=i-13): evac + store ----
            w = i - 13
            if 0 <= w < nw:
                zp = zp_t[w]
                zs = pzs.tile([128, 256], F16, tag="zs", name="zs")
                nc.scalar.copy(zs[:], zp[:, 0:256])
                dst = AP(zt.tensor, w * 256 * 128,
                         [[128, 128], [128 * 128, 2], [1, 128]])
                nc.sync.dma_start(dst, zs[:])
                del zp_t[w]
    nc.compile()
    return nc


@functools.lru_cache(maxsize=2)
def _get_program(nw: int):
    return _build_program(nw)


def _im2win(x: np.ndarray) -> np.ndarray:
    """[B,T,H,W,C] -> [882,128,256] windows, flat order f = i_n*B + i_b."""
    s = x.strides
    xs = np.lib.stride_tricks.as_strided(
        x,
        shape=(B, T, NHW, PATCH, NHW, PATCH, C),
        strides=(s[0], s[1], STEP * s[2], s[2], STEP * s[3], s[3], s[4]),
    )
    w = xs.transpose(2, 4, 0, 1, 3, 5, 6)  # [iH,iW,b,t,p,q,c]
    return np.ascontiguousarray(w.reshape(NHW * NHW * B, L, C))


def _overlap_add(zwin: np.ndarray, bo: np.ndarray) -> np.ndarray:
    """[882,128,256] window outputs -> [B,T,H,W,C] with count-normalize + bo."""
    th = np.arange(NHW) * STEP
    z = zwin.reshape(B, NHW, NHW, T, PATCH, PATCH, MID)  # [b,iH,iW,t,p,q,c]
    acc = np.zeros((B, T, H, W, MID), np.float32)
    count = np.zeros((H, W), np.float32)
    for p in range(PATCH):
        rid = (th + p)[:, None]
        for q in range(PATCH):
            cid = (th + q)[None, :]
            acc[:, :, rid, cid, :] += z[:, :, :, :, p, q, :].transpose(0, 3, 1, 2, 4)
            count[rid, cid] += 1.0
    out = acc / count[None, None, :, :, None] + bo[None, None, None, None, :]
    return out


LAST_RESULT = None


def kernel(x, ln_g, ln_b, Wq, Wk, Wv, Wo, bo):
    x = np.asarray(x, np.float32)
    ln_g = np.asarray(ln_g, np.float32)
    ln_b = np.asarray(ln_b, np.float32)
    assert np.allclose(ln_b, 0.0), "kernel folds ln_g into weights; ln_b must be 0"
    # Fold LN gamma into the input side of Wq/Wk/Wv.
    wq_t = np.ascontiguousarray((np.asarray(Wq, np.float32) * ln_g).T.astype(np.float16))
    wk_t = np.ascontiguousarray((np.asarray(Wk, np.float32) * ln_g).T.astype(np.float16))
    wv_t = np.ascontiguousarray((np.asarray(Wv, np.float32) * ln_g).T.astype(np.float16))
    wo_t = np.ascontiguousarray(np.asarray(Wo, np.float32).T.astype(np.float16))
    ones1 = np.ones((128, 32), np.float16)
    ident = np.eye(128, dtype=np.float16)
    maskq = np.zeros((128, 1024), np.float16)
    for h in range(HEADS):
        j = h % 4
        maskq[32 * j:32 * j + 32, h * 128:(h + 1) * 128] = 1.0

    win = _im2win(x)                              # [882, 128, 256]
    pad = NCORES * NW - NWIN                      # 14
    winp = np.concatenate([win, np.zeros((pad, L, C), np.float32)], 0)
    shards = winp.reshape(NCORES, NW * L, C)

    nc = _get_program(NW)
    trace = bool(int(__import__("os").environ.get("KERNEL_TRACE", "0")))
    in_maps = []
    for i in range(NCORES):
        in_maps.append({
            "xw": np.ascontiguousarray(shards[i]).astype(np.float16),
            "wq": wq_t, "wk": wk_t, "wv": wv_t, "wo": wo_t,
            "ones1": ones1, "ident": ident, "maskq": maskq,
        })
    res = run_bass_kernel_spmd(nc, in_maps, core_ids=list(range(NCORES)),
                               trace=trace)
    global LAST_RESULT
    LAST_RESULT = res
    zts = [np.asarray(res.results[i]["zt"], np.float32).reshape(NW, 2, 128, 128)
           for i in range(NCORES)]
    # zt rows: w*256 + c_out, cols l  ->  Z_w[l, c] = zt[w, :, :, l]
    zall = np.concatenate(zts, 0)                 # [896, 2, 128, 128]
    zwin = zall.reshape(NCORES * NW, MID, L).transpose(0, 2, 1)[:NWIN]
    return _overlap_add(np.ascontiguousarray(zwin), np.asarray(bo, np.float32))
